# revision 5
# baseline (speedup 1.0000x reference)
"""Trainium2 Bass kernel for nn_Attention_68882685494025 (BEiT-style windowed
attention with relative position bias).

Strategy: data-parallel over batch (B=64 -> 8 cores x 8). Per core, batches are
processed in 4 pairs (394 tokens) through a fused pipeline:
  A) qkv projection: q,k produced transposed [j, t] (fp32r matmuls), v produced
     natural [t, j] with an interleaved ones-column (for softmax denominators).
  B) scores computed transposed S.T[m, n] = kT.T @ qT (bf16), rel-pos bias added
     via an identity-matmul accumulation into the same PSUM bank.
  C) E = exp(0.125 * psum) on ACT; O.T = [v | 1].T @ E gives output AND row sums.
  D) normalize via DVE reciprocal + gpsimd partition-broadcast + multiply, then
     proj matmul (fp32r) with proj bias added via a K=1 ones matmul.
Biases: q/k biases added on the ACT evacuation (per-partition bias), v_bias and
proj_b folded on host into pb_eff = proj_b + proj_w @ v_bias (exact).
"""

import os
import sys

sys.path.insert(0, "/opt/trn_rl_repo")

import numpy as np
import ml_dtypes

import concourse.bass as bass
import concourse.mybir as mybir
import concourse.tile as tile
from concourse import bacc
from concourse.bass_utils import run_bass_kernel_spmd

dt = mybir.dt
AF = mybir.ActivationFunctionType
ALU = mybir.AluOpType

WH, WW = 14, 14
H = 12
D = 64
N = WH * WW + 1            # 197
C = 768
B_FULL = 64
N_CORES = 8
B_SH = B_FULL // N_CORES   # 8 batches per core
T = B_SH * N               # 1576 tokens per core
NPAIR = 4                  # pairs of batches per core
TP = 2 * N                 # 394 tokens per pair
NUM_REL = (2 * WH - 1) * (2 * WW - 1) + 3

# ragged 128-chunks of a 394-token pair
PAIR_CHUNKS = [(0, 128), (128, 128), (256, 128), (384, 10)]
# m (key) tiles of one batch
M_TILES = [(0, 128), (128, 69)]


def _gen_rel_pos_index(wh, ww):
    area = wh * ww
    coords = np.stack(np.meshgrid(np.arange(wh), np.arange(ww), indexing="ij"))
    cf = coords.reshape(2, -1)
    rel = cf[:, :, None] - cf[:, None, :]
    rel = rel.transpose(1, 2, 0).copy()
    rel[..., 0] += wh - 1
    rel[..., 1] += ww - 1
    rel[..., 0] *= 2 * ww - 1
    nrd = (2 * wh - 1) * (2 * ww - 1) + 3
    idx = np.zeros((area + 1, area + 1), dtype=np.int64)
    idx[1:, 1:] = rel.sum(-1)
    idx[0, 0:] = nrd - 3
    idx[0:, 0] = nrd - 2
    idx[0, 0] = nrd - 1
    return idx


REL_IDX = _gen_rel_pos_index(WH, WW)  # (197, 197)

_CACHED = None


def _build():
    nc = bacc.Bacc(None)

    x_d = nc.dram_tensor("x_sh", [T, C], dt.float32r, kind="ExternalInput")
    wqk_d = nc.dram_tensor("wqk", [128, 6, 2 * C], dt.float32r, kind="ExternalInput")
    wv_d = nc.dram_tensor("wv", [128, 6, C], dt.float32r, kind="ExternalInput")
    pw_d = nc.dram_tensor("pw", [128, 6, C], dt.float32r, kind="ExternalInput")
    rpb_d = nc.dram_tensor("rpb8", [128, H, 2, N], dt.bfloat16, kind="ExternalInput")
    qkb_d = nc.dram_tensor("qkb", [128, 12], dt.float32, kind="ExternalInput")
    pbe_d = nc.dram_tensor("pbe", [1, C], dt.float32r, kind="ExternalInput")
    ones_d = nc.dram_tensor("ones1", [1, 128], dt.float32r, kind="ExternalInput")
    idT_d = nc.dram_tensor("identT", [128, 128], dt.float32r, kind="ExternalInput")
    idB_d = nc.dram_tensor("identB", [128, 128], dt.bfloat16, kind="ExternalInput")
    y_d = nc.dram_tensor("y_sh", [T, C], dt.float32, kind="ExternalOutput")

    with tile.TileContext(nc) as tc:
        with (
            tc.tile_pool(name="const", bufs=1) as constp,
            tc.tile_pool(name="xstage", bufs=3) as xstagep,
            tc.tile_pool(name="xt", bufs=2) as xtp,
            tc.tile_pool(name="qkt", bufs=2) as qktp,
            tc.tile_pool(name="vp", bufs=6) as vp,
            tc.tile_pool(name="ep", bufs=6) as ep,
            tc.tile_pool(name="otp", bufs=2) as otp,
            tc.tile_pool(name="yp", bufs=4) as yp,
            tc.tile_pool(name="rp", bufs=4) as rp,
            tc.tile_pool(name="rbp", bufs=4) as rbp,
            tc.tile_pool(name="psA", bufs=4, space="PSUM") as psA,
            tc.tile_pool(name="psSO", bufs=4, space="PSUM") as psSO,
        ):
            # ---- resident constants / weights ----
            wqk = constp.tile([128, 6, 2 * C], dt.float32r)
            nc.sync.dma_start(wqk[:], wqk_d[:])
            wv = constp.tile([128, 6, C], dt.float32r)
            nc.sync.dma_start(wv[:], wv_d[:])
            pw = constp.tile([128, 6, C], dt.float32r)
            nc.sync.dma_start(pw[:], pw_d[:])
            rpb = constp.tile([128, H, 2, N], dt.bfloat16)
            nc.sync.dma_start(rpb[:], rpb_d[:])
            qkb = constp.tile([128, 12], dt.float32)
            nc.sync.dma_start(qkb[:], qkb_d[:])
            pbe = constp.tile([1, C], dt.float32r)
            nc.sync.dma_start(pbe[:], pbe_d[:])
            ones1 = constp.tile([1, 128], dt.float32r)
            nc.sync.dma_start(ones1[:], ones_d[:])
            idT = constp.tile([128, 128], dt.float32r)
            nc.sync.dma_start(idT[:], idT_d[:])
            idB = constp.tile([128, 128], dt.bfloat16)
            nc.sync.dma_start(idB[:], idB_d[:])

            # proj-bias broadcast tile [128, 768] f32, built once via a K=1
            # ones-matmul so the per-chunk y evacuation is a single DVE add.
            pbb = constp.tile([128, C], dt.float32)
            for eh in range(2):
                pb_ps = psA.tile([128, 384], dt.float32, tag="big")
                nc.tensor.matmul(
                    pb_ps[:],
                    ones1[0:1, :],
                    pbe[0:1, eh * 384 : (eh + 1) * 384],
                    start=True,
                    stop=True,
                )
                nc.vector.tensor_copy(pbb[:, eh * 384 : (eh + 1) * 384], pb_ps[:])

            for pair in range(NPAIR):
                t_base = pair * TP

                # ---- x transpose: xT[c, t] for this pair ----
                xT = xtp.tile([128, 6, TP], dt.float32r, tag="xt")
                for t0, tn in PAIR_CHUNKS:
                    xa = xstagep.tile([128, C], dt.float32r, tag="xa")
                    nc.sync.dma_start(
                        xa[0:tn, :], x_d[t_base + t0 : t_base + t0 + tn, :]
                    )
                    for c in range(6):
                        pt = psA.tile([128, 128], dt.float32r, tag="big")
                        nc.tensor.transpose(
                            pt[0:128, 0:tn],
                            xa[0:tn, c * 128 : (c + 1) * 128],
                            idT[0:tn, 0:tn],
                        )
                        nc.scalar.copy(xT[:, c, t0 : t0 + tn], pt[:, 0:tn])

                # ---- stage A-qk: qkT[j, t] bf16, with q/k bias ----
                qkT = qktp.tile([128, 12, TP], dt.bfloat16, tag="qkt")
                for j in range(12):
                    pa = psA.tile([128, TP], dt.float32, tag="big")
                    for c in range(6):
                        nc.tensor.matmul(
                            pa[:],
                            wqk[:, c, j * 128 : (j + 1) * 128],
                            xT[:, c, :],
                            start=(c == 0),
                            stop=(c == 5),
                        )
                    nc.scalar.activation(
                        qkT[:, j, :], pa[:], AF.Identity, bias=qkb[:, j : j + 1]
                    )

                # ---- stage A-v: v natural [t, j] bf16 with ones columns ----
                vtiles = []  # [bi][mt] -> tile
                for bi in range(2):
                    row = []
                    for mt, (m0, mn) in enumerate(M_TILES):
                        vt = vp.tile([128, H * 65], dt.bfloat16, tag="vt")
                        nc.vector.memset(
                            vt[:].rearrange("p (h c) -> p h c", c=65)[:, :, 64:65],
                            1.0,
                        )
                        for eh in range(2):
                            pv = psA.tile([128, 384], dt.float32, tag="big")
                            for c in range(6):
                                nc.tensor.matmul(
                                    pv[0:mn, :],
                                    xT[:, c, bi * N + m0 : bi * N + m0 + mn],
                                    wv[:, c, eh * 384 : (eh + 1) * 384],
                                    start=(c == 0),
                                    stop=(c == 5),
                                )
                            nc.vector.tensor_copy(
                                vt[0:mn].rearrange("p (h c) -> p h c", c=65)[
                                    :, eh * 6 : (eh + 1) * 6, 0:64
                                ],
                                pv[0:mn, :].rearrange("p (h d) -> p h d", d=64),
                            )
                        row.append(vt)
                    vtiles.append(row)

                # ---- stages B/C per (batch-in-pair, head) ----
                OT = otp.tile([128, 6, TP], dt.float32r, tag="ot")
                for bi in range(2):
                    for h in range(12):
                        jq = h // 2
                        jk = 6 + h // 2
                        po = (h % 2) * 64
                        etiles = []
                        for mt, (m0, mn) in enumerate(M_TILES):
                            ps = psSO.tile([128, N], dt.float32, tag="so")
                            nc.tensor.matmul(
                                ps[0:mn, :],
                                qkT[po : po + 64, jk, bi * N + m0 : bi * N + m0 + mn],
                                qkT[po : po + 64, jq, bi * N : (bi + 1) * N],
                                start=True,
                                stop=False,
                                skip_group_check=True,
                            )
                            nc.tensor.matmul(
                                ps[0:mn, :],
                                idB[0:mn, 0:mn],
                                rpb[0:mn, h, mt, :],
                                start=False,
                                stop=True,
                                skip_group_check=True,
                            )
                            et = ep.tile([128, N], dt.bfloat16, tag="et")
                            nc.scalar.activation(
                                et[0:mn, :], ps[0:mn, :], AF.Exp, bias=0.0, scale=0.125
                            )
                            etiles.append(et)
                        po_t = psSO.tile([128, N], dt.float32, tag="so")
                        nc.tensor.matmul(
                            po_t[0:65, :],
                            vtiles[bi][0][:, h * 65 : (h + 1) * 65],
                            etiles[0][0:128, :],
                            start=True,
                            stop=False,
                        )
                        nc.tensor.matmul(
                            po_t[0:65, :],
                            vtiles[bi][1][0:69, h * 65 : (h + 1) * 65],
                            etiles[1][0:69, :],
                            start=False,
                            stop=True,
                        )
                        r1 = rp.tile([1, N], dt.float32, tag="r1")
                        nc.vector.reciprocal(r1[:], po_t[64:65, :])
                        rb = rbp.tile([64, N], dt.float32, tag="rb")
                        nc.gpsimd.partition_broadcast(rb[:], r1[:])
                        nc.vector.tensor_tensor(
                            OT[po : po + 64, h // 2, bi * N : (bi + 1) * N],
                            po_t[0:64, :],
                            rb[:],
                            ALU.mult,
                        )

                # ---- stage D: y = OT.T @ projwT + pb_eff ----
                for t0, tn in PAIR_CHUNKS:
                    for eh in range(2):
                        pd = psA.tile([128, 384], dt.float32, tag="big")
                        for f in range(6):
                            nc.tensor.matmul(
                                pd[0:tn, :],
                                OT[:, f, t0 : t0 + tn],
                                pw[:, f, eh * 384 : (eh + 1) * 384],
                                start=(f == 0),
                                stop=(f == 5),
                            )
                        yt = yp.tile([128, 384], dt.float32, tag="yt")
                        nc.vector.tensor_tensor(
                            yt[0:tn, :],
                            pd[0:tn, :],
                            pbb[0:tn, eh * 384 : (eh + 1) * 384],
                            ALU.add,
                        )
                        nc.sync.dma_start(
                            y_d[
                                t_base + t0 : t_base + t0 + tn,
                                eh * 384 : (eh + 1) * 384,
                            ],
                            yt[0:tn, :],
                        )

    nc.finalize()
    return nc


def _host_prep(x, qkv_w, q_bias, k_bias, v_bias, rel_table, proj_w, proj_b):
    f32 = np.float32
    bf16 = ml_dtypes.bfloat16

    wqk_T = np.ascontiguousarray(qkv_w[: 2 * C].T)  # [c, j]
    wv_T = np.ascontiguousarray(qkv_w[2 * C :].T)   # [c, j]
    pw_T = np.ascontiguousarray(proj_w.T)           # [f, e]

    wqk_h = np.ascontiguousarray(
        wqk_T.reshape(6, 128, 2 * C).transpose(1, 0, 2)
    ).astype(f32)
    wv_h = np.ascontiguousarray(wv_T.reshape(6, 128, C).transpose(1, 0, 2)).astype(f32)
    pw_h = np.ascontiguousarray(pw_T.reshape(6, 128, C).transpose(1, 0, 2)).astype(f32)

    rpb_full = rel_table[REL_IDX]                   # [n, m, H]
    R8T = 8.0 * rpb_full.transpose(2, 1, 0)         # [H, m, n]
    rpb_h = np.zeros((128, H, 2, N), dtype=bf16)
    for mt, (m0, mn) in enumerate(M_TILES):
        rpb_h[:mn, :, mt, :] = R8T[:, m0 : m0 + mn, :].transpose(1, 0, 2).astype(bf16)

    qkb_h = np.ascontiguousarray(
        np.concatenate([q_bias, k_bias]).reshape(12, 128).T
    ).astype(f32)
    pbe_h = (proj_b + proj_w @ v_bias).reshape(1, C).astype(f32)
    ones_h = np.ones((1, 128), f32)
    idT_h = np.eye(128, dtype=f32)
    idB_h = np.eye(128, dtype=bf16)

    shared = {
        "wqk": wqk_h,
        "wv": wv_h,
        "pw": pw_h,
        "rpb8": rpb_h,
        "qkb": qkb_h,
        "pbe": pbe_h,
        "ones1": ones_h,
        "identT": idT_h,
        "identB": idB_h,
    }
    x_sh = np.ascontiguousarray(x.reshape(N_CORES, T, C)).astype(f32)
    return [dict(shared, x_sh=x_sh[i]) for i in range(N_CORES)]


def kernel(**inputs):
    global _CACHED
    if _CACHED is None:
        _CACHED = _build()
    nc = _CACHED

    in_maps = _host_prep(
        np.asarray(inputs["x"], np.float32),
        np.asarray(inputs["qkv_w"], np.float32),
        np.asarray(inputs["q_bias"], np.float32),
        np.asarray(inputs["k_bias"], np.float32),
        np.asarray(inputs["v_bias"], np.float32),
        np.asarray(inputs["rel_table"], np.float32),
        np.asarray(inputs["proj_w"], np.float32),
        np.asarray(inputs["proj_b"], np.float32),
    )

    trace = bool(int(os.environ.get("BASS_KERNEL_TRACE", "0")))
    res = run_bass_kernel_spmd(
        nc, in_maps, core_ids=list(range(N_CORES)), trace=trace
    )
    if trace and res.exec_time_ns is not None:
        print(f"HW exec time: {res.exec_time_ns} ns")
        if res.instructions_and_trace is not None:
            print(f"trace: {res.instructions_and_trace[1]}")

    y = np.stack([r["y_sh"] for r in res.results], axis=0)  # [8, T, C]
    return np.ascontiguousarray(y.reshape(B_FULL, N, C))
